# revision 4
# baseline (speedup 1.0000x reference)
"""Compact Bilinear Pooling (B=16, C=512, HW=196, OUT=8192) on 8 TRN2 NeuronCores.

Math (same as baseline): per batch b,
    Rhat_b[f] = sum_hw U1[hw,f] * U2[hw,f],  U_j = x_hw @ A_j,
where row c of A_j is a DFT-phase row selected by the count-sketch index
h_j[c] (sign folded as a half-turn phase offset), followed by a 64x128
Cooley-Tukey irfft, signed-sqrt and L2 normalization.

v4: tables generated on device from 8KB metadata (as v2), and the
projection runs TRANSPOSED: the DFT table chunk is the stationary operand
[c=128, k=128] and x is the moving operand [c=128, (b,hw)=392], so each
table is streamed once over the whole batch*hw axis instead of once per
(batch, hw-block).  U comes out as [k, b*hw]; the spatial reduction is a
per-chunk DVE row-reduce into a [k=128, chunk] spectrum (no band-scatter
matmuls), and a single copy+PE-transpose per batch rebuilds the [f1, f2]
spectrum tile for the irfft.  The irfft's E128 factor absorbs the
resulting f1 row-interleave via a permuted on-device generation.
fp16 hi/lo pair split throughout (baseline accuracy ~1e-4).

Host does only sharding/reformatting. Data-parallel over batch, 2 batches
per core, no collectives.
"""

import numpy as np

B, C, HW, N = 16, 512, 196, 8192
NF = N // 2 + 1          # 4097 rfft bins
CHUNK = 128              # frequency bins per chunk (k on partitions)
NCHUNK = 33              # 32 full chunks + Nyquist chunk
NCORES = 8
BPC = B // NCORES        # batches per core
BHW = BPC * HW           # 392 moving columns
EPS_SQRT = 1e-5
EPS_NORM = 1e-12

_COMPILED = {}


def _build_meta(sketch1, sketch2):
    """Extract (h, phase-offset) int32 metadata, packed [128, 16]."""
    def hs(sk):
        sk = np.asarray(sk)
        h = np.abs(sk).argmax(axis=1).astype(np.int64)
        s = sk[np.arange(C), h]
        off = (N // 2) * (s < 0)
        return (h.reshape(4, 128).T.astype(np.int32),
                off.reshape(4, 128).T.astype(np.int32))

    h1, o1 = hs(sketch1)
    h2, o2 = hs(sketch2)
    return np.ascontiguousarray(np.concatenate([h1, o1, h2, o2], axis=1))


def _build_program():
    import concourse.mybir as mybir
    import concourse.tile as tile
    from concourse import bacc

    f32 = mybir.dt.float32
    f16 = mybir.dt.float16
    i32 = mybir.dt.int32
    AF = mybir.ActivationFunctionType
    OP = mybir.AluOpType

    PI = float(np.pi)
    MASK = N - 1

    nc = bacc.Bacc("TRN2", target_bir_lowering=False, debug=False,
                   num_devices=NCORES)

    # x packed host-side as [c_in_kc=128, kc=4, b=BPC, hw=HW]
    xin = nc.dram_tensor("x", [128, 4, BPC, HW], f32,
                         kind="ExternalInput").ap()
    meta_in = nc.dram_tensor("meta", [128, 16], i32, kind="ExternalInput").ap()
    out = nc.dram_tensor("out", [BPC, 128, 64], f32, kind="ExternalOutput").ap()

    with tile.TileContext(nc) as tc:
        with (
            tc.tile_pool(name="xpool", bufs=1) as xpool,
            tc.tile_pool(name="gpool", bufs=1) as gpool,
            tc.tile_pool(name="apool", bufs=3) as apool,
            tc.tile_pool(name="cpool", bufs=1) as cpool,
            tc.tile_pool(name="hpool", bufs=3) as hpool,
            tc.tile_pool(name="small", bufs=2) as small,
            tc.tile_pool(name="upsum", bufs=1, space="PSUM") as upsum,
            tc.tile_pool(name="tpsum", bufs=1, space="PSUM") as tpsum,
            tc.tile_pool(name="npsum", bufs=1, space="PSUM") as npsum,
            tc.tile_pool(name="spsum", bufs=1, space="PSUM") as spsum,
        ):
            # ---- load x (one DMA), split into fp16 pair: x = xh + xl ----
            xt = xpool.tile([128, 4, BHW], f32, tag="xf32")
            nc.sync.dma_start(xt[:], xin[:])
            xh = xpool.tile([128, 4, BHW], f16, tag="x16h", name="x16h")
            nc.vector.tensor_copy(xh[:], xt[:])
            xlf = xpool.tile([128, 4, BHW], f32, tag="xlf")
            nc.vector.tensor_tensor(xlf[:], xt[:], xh[:], op=OP.subtract)
            xl = xpool.tile([128, 4, BHW], f16, tag="x16l", name="x16l")
            nc.vector.tensor_copy(xl[:], xlf[:])

            # ---- shared scalar constants ----
            mpi = cpool.tile([128, 1], f32, tag="mpi", name="mpi")
            nc.gpsimd.memset(mpi[:], -PI)
            ones = cpool.tile([128, 128], f32, tag="ones", name="ones")
            nc.gpsimd.memset(ones[:], 1.0)
            eps_b = cpool.tile([128, 1], f32, tag="eps_b", name="eps_b")
            nc.gpsimd.memset(eps_b[:], EPS_SQRT)
            eps_n = cpool.tile([128, 1], f32, tag="eps_n", name="eps_n")
            nc.gpsimd.memset(eps_n[:], float(N) * EPS_SQRT)

            # ---- irfft constants, generated on device ----
            ct = {}

            def gen_trig(key, parts, cols, per, add, perm=False):
                """tile[p, j] = sin-act of ((rowval(p)*j + add) mod per).

                perm=True uses the interleaved row order rowval(p) =
                2*(p%32) + p//32 that the spectrum transpose produces.
                """
                pio = cpool.tile([parts, cols], i32, tag=f"{key}_pio")
                if perm:
                    nc.gpsimd.iota(pio[0:32, :], pattern=[[0, cols]], base=0,
                                   channel_multiplier=2)
                    nc.gpsimd.iota(pio[32:64, :], pattern=[[0, cols]], base=1,
                                   channel_multiplier=2)
                else:
                    nc.gpsimd.iota(pio[:], pattern=[[0, cols]], base=0,
                                   channel_multiplier=1)
                jio = cpool.tile([parts, cols], i32, tag=f"{key}_jio")
                nc.gpsimd.iota(jio[:], pattern=[[1, cols]], base=0,
                               channel_multiplier=0)
                phi = cpool.tile([parts, cols], i32, tag=f"{key}_phi")
                nc.gpsimd.tensor_tensor(phi[:], pio[:], jio[:], op=OP.mult)
                nc.vector.tensor_scalar(phi[:], phi[:], add, None, op0=OP.add)
                nc.vector.tensor_scalar(phi[:], phi[:], per - 1, None,
                                        op0=OP.bitwise_and)
                ph = cpool.tile([parts, cols], f32, tag=f"{key}_ph")
                nc.vector.tensor_copy(ph[:], phi[:])
                t = cpool.tile([parts, cols], f32, tag=key, name=key)
                nc.scalar.activation(t[:], ph[:], AF.Sin, bias=mpi[:parts, :],
                                     scale=2.0 * PI / per)
                ct[key] = t

            gen_trig("e128c", 64, 128, 128, 96, perm=True)    # cos
            gen_trig("e128s", 64, 128, 128, 64, perm=True)    # +sin
            gen_trig("e128sn", 64, 128, 128, 0, perm=True)    # -sin
            gen_trig("twc", 64, 128, N, 3 * N // 4)  # cos(2pi p j/N)
            gen_trig("tws", 64, 128, N, N // 2)      # +sin
            gen_trig("e64c", 64, 64, 64, 48)         # cos(2pi p j/64)
            gen_trig("e64sn", 64, 64, 64, 0)         # -sin

            # identity [64, 64] for PE transpose
            idp = cpool.tile([64, 64], i32, tag="id_p")
            nc.gpsimd.iota(idp[:], pattern=[[0, 64]], base=0,
                           channel_multiplier=1)
            idj = cpool.tile([64, 64], i32, tag="id_j")
            nc.gpsimd.iota(idj[:], pattern=[[1, 64]], base=0,
                           channel_multiplier=0)
            ident = cpool.tile([64, 64], f32, tag="ident", name="ident")
            nc.vector.tensor_tensor(ident[:], idp[:], idj[:], op=OP.is_equal)

            ones_col = cpool.tile([128, 1], f32, tag="ones_col")
            nc.gpsimd.memset(ones_col[:], 1.0)
            ct["ones_col"] = ones_col
            ones_row = cpool.tile([1, 128], f32, tag="ones_row")
            nc.gpsimd.memset(ones_row[:], 1.0)
            ct["ones_row"] = ones_row
            mones_row = cpool.tile([1, 128], f32, tag="mones_row")
            nc.gpsimd.memset(mones_row[:], -1.0)
            ct["mones_row"] = mones_row
            alt = cpool.tile([1, 128], f32, tag="alt_row", name="alt_row")
            aio = cpool.tile([1, 128], i32, tag="alt_io")
            nc.gpsimd.iota(aio[:], pattern=[[1, 128]], base=0,
                           channel_multiplier=0)
            nc.vector.tensor_scalar(aio[:], aio[:], 1, None,
                                    op0=OP.bitwise_and)
            nc.vector.tensor_copy(alt[:], aio[:])
            nc.vector.tensor_scalar(alt[:], alt[:], -2.0, 1.0,
                                    op0=OP.mult, op1=OP.add)
            ct["alt_row"] = alt

            # ---- sketch metadata -> int32 phase-recurrence state ----
            meta = gpool.tile([128, 16], i32, tag="meta", name="meta")
            nc.sync.dma_start(meta[:], meta_in[:])
            metaf = gpool.tile([128, 16], f32, tag="metaf", name="metaf")
            nc.vector.tensor_copy(metaf[:], meta[:])
            kio = gpool.tile([128, 4, 128], i32, tag="kio")
            nc.gpsimd.iota(kio[:], pattern=[[0, 4], [1, 128]], base=0,
                           channel_multiplier=0)
            ph = [None, None]    # running phase int32 [128, 4, 128] per sketch
            dht = [None, None]   # int32 phase step per chunk
            for j in range(2):
                bf = gpool.tile([128, 128], f32, tag="bf", name=f"bf{j}")
                hti = gpool.tile([128, 4, 128], i32, tag=f"hti{j}")
                oti = gpool.tile([128, 4, 128], i32, tag=f"oti{j}")
                for kc in range(4):
                    hc = 8 * j + kc
                    nc.vector.tensor_scalar(bf[:], ones[:],
                                            metaf[:, hc:hc + 1], None,
                                            op0=OP.mult)
                    nc.vector.tensor_copy(hti[:, kc], bf[:])
                    nc.vector.tensor_scalar(bf[:], ones[:],
                                            metaf[:, hc + 4:hc + 5], None,
                                            op0=OP.mult)
                    nc.vector.tensor_copy(oti[:, kc], bf[:])
                dh = gpool.tile([128, 4, 128], i32, tag=f"dht{j}",
                                name=f"dht{j}")
                nc.vector.tensor_scalar(dh[:], hti[:], 128, None, op0=OP.mult)
                nc.vector.tensor_scalar(dh[:], dh[:], MASK, None,
                                        op0=OP.bitwise_and)
                pc = gpool.tile([128, 4, 256], i32, tag=f"phcat{j}",
                                name=f"phcat{j}")
                p0 = pc[:, :, 128:256]
                nc.gpsimd.tensor_tensor(p0, kio[:], hti[:], op=OP.mult)
                nc.gpsimd.tensor_tensor(p0, p0, oti[:], op=OP.add)
                nc.vector.tensor_scalar(p0, p0, MASK, None,
                                        op0=OP.bitwise_and)
                ph[j] = pc
                dht[j] = dh

            # spectrum accumulators: S[b][0]=re, S[b][1]=im, [k=128, chunk]
            S = [[gpool.tile([128, 32], f32, tag=f"S_{b}_{p}",
                             name=f"S_{b}_{p}") for p in range(2)]
                 for b in range(BPC)]
            r16_sb = gpool.tile([1, BPC], f32, tag="r16_sb", name="r16_sb")

            # ---- main loop over frequency chunks ----
            for chunk in range(NCHUNK):
                # -- generate this chunk's fp16 hi/lo tables on device --
                achh = []
                achl = []
                for j in range(2):
                    run = ph[j][:, :, 128:256]
                    pre = ph[j][:, :, 0:128]
                    if chunk > 0:
                        nc.gpsimd.tensor_tensor(run, run, dht[j][:],
                                                op=OP.add)
                        nc.vector.tensor_scalar(run, run, MASK, None,
                                                op0=OP.bitwise_and)
                    nc.vector.tensor_scalar(pre, run, 3 * N // 4,
                                            None, op0=OP.add)
                    nc.vector.tensor_scalar(pre, pre, MASK, None,
                                            op0=OP.bitwise_and)
                    v = hpool.tile([128, 4, 256], f32, tag=f"v{j}")
                    nc.scalar.activation(v[:], ph[j][:], AF.Sin, bias=mpi[:],
                                         scale=2.0 * PI / N)
                    th = apool.tile([128, 4, 256], f16, tag=f"achh{j}",
                                    name=f"achh{j}_{chunk}")
                    nc.scalar.copy(th[:], v[:])
                    tl = apool.tile([128, 4, 256], f16, tag=f"achl{j}",
                                    name=f"achl{j}_{chunk}")
                    nc.gpsimd.tensor_tensor(tl[:], v[:], th[:],
                                            op=OP.subtract)
                    achh.append(th)
                    achl.append(tl)

                if chunk == 32:
                    # Nyquist bin f=4096 (re only): stationary = table col 0
                    ny = npsum.tile([1, 2, 512], f32, tag="ny")
                    for j in range(2):
                        first = True
                        for kc in range(4):
                            for lhsT, mv in ((achh[j][:, kc, 0:1], xh),
                                             (achl[j][:, kc, 0:1], xh),
                                             (achh[j][:, kc, 0:1], xl)):
                                nc.tensor.matmul(ny[:, j, 0:BHW], lhsT,
                                                 mv[:, kc, :],
                                                 start=(first and kc == 0),
                                                 stop=(kc == 3
                                                       and mv is xl))
                                first = False
                    nysb = hpool.tile([1, 2, 392], f32, tag="nysb")
                    nc.scalar.copy(nysb[:], ny[:, :, 0:BHW])
                    nyp = hpool.tile([1, 392], f32, tag="nyp")
                    nc.vector.tensor_tensor(nyp[:], nysb[:, 0, :],
                                            nysb[:, 1, :], op=OP.mult)
                    for b in range(BPC):
                        nc.vector.reduce_sum(r16_sb[:, b:b + 1],
                                             nyp[:, b * HW:(b + 1) * HW],
                                             axis=mybir.AxisListType.X)
                    continue

                # -- projection: out[k, bhw] accumulated per region --
                # regions: 0=U1re, 1=U1im, 2=U2re, 3=U2im (bank-aligned 512)
                u12 = upsum.tile([128, 4, 512], f32, tag="u12")
                for j in range(2):
                    for half in range(2):  # 0=re, 1=im
                        reg = 2 * j + half
                        ksl = slice(128 * half, 128 * half + 128)
                        for kc in range(4):
                            hi = achh[j][:, kc, ksl]
                            lo = achl[j][:, kc, ksl]
                            nc.tensor.matmul(u12[:, reg, 0:BHW], hi,
                                             xh[:, kc, :],
                                             start=(kc == 0), stop=False)
                            nc.tensor.matmul(u12[:, reg, 0:BHW], hi,
                                             xl[:, kc, :],
                                             start=False, stop=False)
                            nc.tensor.matmul(u12[:, reg, 0:BHW], lo,
                                             xh[:, kc, :],
                                             start=False, stop=(kc == 3))

                # -- stage to SBUF, hadamard, per-batch row-reduce --
                usb = hpool.tile([128, 4, 392], f32, tag="usb")
                nc.scalar.copy(usb[:], u12[:, :, 0:BHW])
                t1 = hpool.tile([128, 392], f32, tag="t1")
                t2 = hpool.tile([128, 392], f32, tag="t2")
                hre = hpool.tile([128, 392], f32, tag="hre")
                him = hpool.tile([128, 392], f32, tag="him")
                nc.vector.tensor_tensor(t1[:], usb[:, 0, :], usb[:, 2, :],
                                        op=OP.mult)
                nc.vector.tensor_tensor(t2[:], usb[:, 1, :], usb[:, 3, :],
                                        op=OP.mult)
                nc.vector.tensor_tensor(hre[:], t1[:], t2[:], op=OP.subtract)
                nc.vector.tensor_tensor(t1[:], usb[:, 0, :], usb[:, 3, :],
                                        op=OP.mult)
                nc.vector.tensor_tensor(t2[:], usb[:, 1, :], usb[:, 2, :],
                                        op=OP.mult)
                nc.vector.tensor_tensor(him[:], t1[:], t2[:], op=OP.add)
                for b in range(BPC):
                    nc.vector.reduce_sum(S[b][0][:, chunk:chunk + 1],
                                         hre[:, b * HW:(b + 1) * HW],
                                         axis=mybir.AxisListType.X)
                    nc.vector.reduce_sum(S[b][1][:, chunk:chunk + 1],
                                         him[:, b * HW:(b + 1) * HW],
                                         axis=mybir.AxisListType.X)

            # ---- per batch: rebuild [f1, f2] spectrum, irfft, tail ----
            for b in range(BPC):
                xr = small.tile([64, 64], f32, tag="xr")
                xi = small.tile([64, 64], f32, tag="xi")
                for p, dstt in ((0, xr), (1, xi)):
                    comb = small.tile([64, 64], f32, tag="comb")
                    nc.sync.dma_start(comb[:, 0:32], S[b][p][0:64, :])
                    nc.sync.dma_start(comb[:, 32:64], S[b][p][64:128, :])
                    tp = tpsum.tile([64, 64], f32, tag="tp")
                    nc.tensor.transpose(tp[:], comb[:], ident[:])
                    nc.scalar.copy(dstt[:], tp[:])
                r16_b = small.tile([1, 1], f32, tag="r16_b")
                nc.scalar.copy(r16_b[:], r16_sb[:, b:b + 1])

                sps = spsum.tile([128, 512], f32, tag="sps")
                yr = sps[0:64, 0:128]
                yi = sps[0:64, 128:256]
                zps = sps[0:128, 256:320]
                tot = sps[0:1, 320:321]
                nrmb = sps[0:128, 352:353]
                cps = sps[0:128, 384:385]

                # c[k1] = -Rhat[0] + (-1)^k1 Rhat[4096]
                nc.tensor.matmul(cps, ct["mones_row"][:], xr[0:1, 0:1],
                                 start=True, stop=False)
                nc.tensor.matmul(cps, ct["alt_row"][:], r16_b[:], start=False,
                                 stop=True)
                c_sb = small.tile([128, 1], f32, tag="c_sb")
                nc.scalar.copy(c_sb[:], cps)

                # stage 1: Y[f2, k1] = sum_f1 Xhat[f1, f2] e^{+2pi i k1 f1/128}
                nc.tensor.matmul(yr, xr[:], ct["e128c"][:], start=True,
                                 stop=False)
                nc.tensor.matmul(yr, xi[:], ct["e128sn"][:], start=False,
                                 stop=True)
                nc.tensor.matmul(yi, xr[:], ct["e128s"][:], start=True,
                                 stop=False)
                nc.tensor.matmul(yi, xi[:], ct["e128c"][:], start=False,
                                 stop=True)

                # twiddle: Y' = Y * (TWc + i TWs), layout [f2=64, k1=128]
                ypr = small.tile([64, 128], f32, tag="ypr")
                ypi = small.tile([64, 128], f32, tag="ypi")
                tt1 = small.tile([64, 128], f32, tag="tt1")
                tt2 = small.tile([64, 128], f32, tag="tt2")
                nc.vector.tensor_tensor(tt1[:], yr, ct["twc"][:], op=OP.mult)
                nc.vector.tensor_tensor(tt2[:], yi, ct["tws"][:], op=OP.mult)
                nc.vector.tensor_tensor(ypr[:], tt1[:], tt2[:], op=OP.subtract)
                nc.vector.tensor_tensor(tt1[:], yr, ct["tws"][:], op=OP.mult)
                nc.vector.tensor_tensor(tt2[:], yi, ct["twc"][:], op=OP.mult)
                nc.vector.tensor_tensor(ypi[:], tt1[:], tt2[:], op=OP.add)

                # stage 2: Z0[k1, k2] = sum_f2 Y'r E64c - Y'i E64s
                nc.tensor.matmul(zps, ypr[:], ct["e64c"][:], start=True,
                                 stop=False)
                nc.tensor.matmul(zps, ypi[:], ct["e64sn"][:], start=False,
                                 stop=True)

                # Z = 2*Z0 + c
                zeff = small.tile([128, 64], f32, tag="zeff")
                nc.vector.tensor_scalar(zeff[:], zps, 2.0, c_sb[:, 0:1],
                                        op0=OP.mult, op1=OP.add)

                # tail: signed sqrt + L2 normalize
                absz = small.tile([128, 64], f32, tag="absz")
                nc.scalar.activation(absz[:], zeff[:], AF.Abs)
                sq = small.tile([128, 64], f32, tag="sq")
                nc.scalar.activation(sq[:], absz[:], AF.Sqrt, bias=eps_b[:])
                sgn = small.tile([128, 64], f32, tag="sgn")
                nc.scalar.activation(sgn[:], zeff[:], AF.Sign)
                ssq = small.tile([128, 64], f32, tag="ssq")
                nc.vector.tensor_tensor(ssq[:], sq[:], sgn[:], op=OP.mult)
                rs = small.tile([128, 1], f32, tag="rs")
                nc.vector.reduce_sum(rs[:], zeff[:], axis=mybir.AxisListType.X,
                                     apply_absolute_value=True)
                nc.tensor.matmul(tot, rs[:], ct["ones_col"][:], start=True,
                                 stop=True)
                nrm = small.tile([1, 1], f32, tag="nrm")
                nc.scalar.activation(nrm[:], tot, AF.Sqrt, bias=eps_n[0:1, :])
                nc.vector.tensor_scalar_max(nrm[:], nrm[:], EPS_NORM)
                nc.vector.reciprocal(nrm[:], nrm[:])
                nc.tensor.matmul(nrmb, ct["ones_row"][:], nrm[:], start=True,
                                 stop=True)
                nrmb_s = small.tile([128, 1], f32, tag="nrmb_s")
                nc.scalar.copy(nrmb_s[:], nrmb)
                fin = small.tile([128, 64], f32, tag="fin")
                nc.vector.tensor_scalar_mul(fin[:], ssq[:], nrmb_s[:])
                nc.sync.dma_start(out[b], fin[:])

    nc.compile()
    return nc


def _get_program():
    if "nc" not in _COMPILED:
        _COMPILED["nc"] = _build_program()
    return _COMPILED["nc"]


def make_in_maps(x, sketch1, sketch2):
    x = np.ascontiguousarray(np.asarray(x), dtype=np.float32)
    meta = _build_meta(sketch1, sketch2)
    xs = x.reshape(B, 4, 128, HW)
    in_maps = []
    for i in range(NCORES):
        blk = xs[i * BPC:(i + 1) * BPC]            # [BPC, kc, 128, HW]
        pk = blk.transpose(2, 1, 0, 3)             # [128, kc, BPC, HW]
        in_maps.append({"x": np.ascontiguousarray(pk), "meta": meta})
    return in_maps


def unshard_out(results):
    outs = np.empty((B, N), dtype=np.float32)
    for i in range(NCORES):
        z = results[i]["out"]  # [BPC, 128, 64]
        for j in range(BPC):
            outs[i * BPC + j] = np.ascontiguousarray(z[j].T).reshape(-1)
    return outs


def kernel(x, sketch1, sketch2):
    from concourse.bass_utils import run_bass_kernel_spmd

    in_maps = make_in_maps(x, sketch1, sketch2)
    nc = _get_program()
    res = run_bass_kernel_spmd(nc, in_maps, core_ids=list(range(NCORES)))
    return unshard_out(res.results)
